# revision 32
# baseline (speedup 1.0000x reference)
"""Trainium2 Bass kernel for nn_CpSae_44014824849572.

Computes the CP-SAE loss. The reference materializes a [1024, 64, 32, 32]
CP-reconstruction `volume` and diffs it against `features`. We instead use

  sum((flat - volume)^2) = sum(flat^2) - 2*sum(flat*volume) + sum(volume^2)

with  sum(flat*volume)[b] = sum_z a[b,z] * T[b,z],
      T[b,z]   = sum_feat flat[b,feat] * KRP[g_b][z,feat]
      KRP[g]   = softplus(freq)⊗softplus(roi1)⊗softplus(roi2)  (rank-1 rows)
      sum(volume^2)[b] = a_b^T M_{g_b} a_b,
      M_g = (Ff Ff^T) ∘ (R1 R1^T) ∘ (R2 R2^T)   (32x32 per group, tiny)

so the only heavy device work is two big contractions over the feature dim:
  zcat[b, 0:64] = flat[b] @ [W1 | W2]          (encoder, 8.6 GFLOP)
  T[b, z]       = flat[b] @ KRP[g_b].T         (4.3 GFLOP)

Instead of materializing KRP (4MB/core in HBM), we exploit its rank-1
structure: within a 128-feature chunk (f1 fixed, 4 r1 values x 32 r2) the
KRP stationary is (R1⊗R2)[g] scaled by Ff[g,z,f1]. The device contracts
against the f1-independent RR = R1⊗R2 stationary (0.5MB, shared by all
cores), accumulating per-f1 partials U[f1][z,b] (each f1 completes
sequentially, so one ping-ponged [32,1024] PSUM tile suffices), then folds
Ff with diag(Ff[g,:,f1]) stationaries: T[z,b] = sum_f1 Ff[g_b,z,f1]*U[f1].
The last two f1 partials ship raw (host folds them) so the tail critical
path is just one PSUM->SBUF narrowing per output.

Distribution: feature-dim sharded across 8 cores (8192 features = 8 f1
values each, all 1024 samples). Samples are group-sorted on the host so
each group's U-matmul sees a contiguous column block. All heavy matmuls are
fp8 with MatmulPerfMode.DoubleRow (two 128-feature chunks contracted per
instruction, 0.5 cycles/row). The ft stream is the bottleneck (~23.3us of
DMA at 360GB/s); consts are sliced into its first pairs and outputs (bf16)
drain behind it. Partial zcat/T are summed on host.
"""
import json

import numpy as np
import ml_dtypes

import concourse.bass as bass
import concourse.mybir as mybir
import concourse.tile as tile
from concourse.bass_utils import run_bass_kernel_spmd

N_CORES = 8
BATCH = 1024
N_FREQS = 64
N_ROIS = 32
Z = 32
N_GROUPS = 16
N_CLASSES = 4
N_FEAT = N_FREQS * N_ROIS * N_ROIS          # 65536
FEAT_PER_CORE = N_FEAT // N_CORES           # 8192
KCHUNKS = FEAT_PER_CORE // 128              # 64
F1_PER_CORE = FEAT_PER_CORE // (N_ROIS * N_ROIS)  # 8
EPSILON = 1e-06
REG_STRENGTH = 1.0
KL_FACTOR = 1.0

F32 = mybir.dt.float32
BF16 = mybir.dt.bfloat16
DT = mybir.dt.float8e4
NPDT = ml_dtypes.float8_e4m3
W_SCALE = 4096.0

_waitfix_counter = [0]


def _split_waits_in_bir(bir: dict) -> int:
    """This container's walrus accepts only ONE sync wait per instruction;
    Tile emits several. Hoist all-but-one wait onto EventSemaphore
    instructions inserted just before, on the same engine."""
    nsplit = 0
    for fn in bir.get("functions", []):
        for blk in fn.get("blocks", []):
            out = []
            for insn in blk.get("instructions", []):
                si = insn.get("sync_info") or {}
                ow = si.get("on_wait") or []
                if len(ow) > 1:
                    for w in ow[:-1]:
                        _waitfix_counter[0] += 1
                        out.append({
                            "debug": insn.get("debug", 0),
                            "engine": insn["engine"],
                            "ins": [],
                            "name": f"{insn['name']}-wsplit{_waitfix_counter[0]}",
                            "opcode": "EventSemaphore",
                            "outs": [],
                            "sync_info": {"on_update": [], "on_wait": [w]},
                        })
                        nsplit += 1
                    si["on_wait"] = [ow[-1]]
                out.append(insn)
            blk["instructions"] = out
    return nsplit


def _install_waitfix():
    import concourse.bass2jax as bass2jax
    import concourse.bass_utils as bass_utils

    if getattr(bass2jax, "_waitfix_installed", False):
        return
    orig = bass_utils.compile_bir_kernel

    def patched(bir_json, tmpdir, neff_name="file.neff"):
        bir = json.loads(bir_json.decode() if isinstance(bir_json, bytes) else bir_json)
        _split_waits_in_bir(bir)
        return orig(json.dumps(bir).encode(), tmpdir, neff_name)

    bass2jax.compile_bir_kernel = patched
    bass_utils.compile_bir_kernel = patched
    bass2jax._waitfix_installed = True


def _softplus(x):
    return np.logaddexp(0.0, x.astype(np.float64)).astype(np.float32)


def _group_blocks(groups_sorted):
    """[(g, c0, c1)] contiguous column block (<=512 wide) per group g."""
    gs = np.asarray(groups_sorted)
    blocks = []
    for g in range(N_GROUPS):
        c0 = int(np.searchsorted(gs, g))
        c1 = int(np.searchsorted(gs, g + 1))
        while c0 < c1:
            ce = min(c0 + 512, c1)
            blocks.append((g, c0, ce))
            c0 = ce
    return blocks


def build_device_program(blocks):
    """One SPMD program (shared by all 8 cores). Per-core inputs:
      flatt [KCHUNKS, 128, BATCH]  — transposed feature slice (group-sorted)
      w     [128, KCHUNKS, 64]     — [W1|W2]*W_SCALE slice, partition-major
      rrt   [128, 8, 16, Z]        — (R1⊗R2) stationary: [ (dr1,r2), blk, g, z ]
      ffi   [128, 2, 16, Z] bf16   — stacked-identity * Ff[g, z, f1(j, p)]
    Outputs (partial sums over this core's features):
      zcat [64, BATCH] f32 — encoder output [W1|W2] partial
      t    [Z, BATCH] f32  — T partial (this core's f1 range)
    """
    nc = bass.Bass()
    flatt = nc.dram_tensor("flatt", [KCHUNKS, 128, BATCH], DT, kind="ExternalInput")
    w = nc.dram_tensor("w", [128, KCHUNKS, 64], DT, kind="ExternalInput")
    rrt = nc.dram_tensor("rrt", [128, 8, N_GROUPS, Z], DT, kind="ExternalInput")
    ffd = nc.dram_tensor("ffd", [Z, F1_PER_CORE, N_GROUPS, Z], BF16,
                         kind="ExternalInput")
    zcat_out = nc.dram_tensor("zcat", [64, BATCH], BF16, kind="ExternalOutput")
    t_out = nc.dram_tensor("t", [Z, BATCH], BF16, kind="ExternalOutput")
    u6_out = nc.dram_tensor("u6", [Z, BATCH], BF16, kind="ExternalOutput")
    u7_out = nc.dram_tensor("u7", [Z, BATCH], BF16, kind="ExternalOutput")

    DR = mybir.MatmulPerfMode.DoubleRow
    NPAIR = KCHUNKS // 2                     # 32

    with tile.TileContext(nc) as tc:
        with (
            tc.tile_pool(name="fpool", bufs=8) as fpool,
            tc.tile_pool(name="const", bufs=1) as const,
            tc.tile_pool(name="opool", bufs=1) as opool,
            tc.tile_pool(name="psum", bufs=1, space="PSUM") as psum,
        ):
            wt = const.tile([128, KCHUNKS, 64], DT, tag="w")
            rrt_sb = const.tile([128, 8, N_GROUPS, Z], DT, tag="rrt")
            ffd_sb = const.tile([Z, F1_PER_CORE, N_GROUPS, Z], BF16, tag="ffd")
            u_sb = [opool.tile([Z, BATCH], BF16, tag=f"u{j}", name=f"u_sb{j}")
                    for j in range(2)]

            zcat_ps = psum.tile([64, BATCH], F32, tag="zcat")
            u_ps = [psum.tile([Z, BATCH], F32, tag=f"u{j}", name=f"u_ps{j}")
                    for j in range(2)]
            t_ps = psum.tile([Z, BATCH], F32, tag="t")

            # --- pipeline: per pair, issue the ft DMA (with const slices
            # interleaved after the first few so PE can start early), then the
            # pair's matmuls. The 8-deep fpool lets DMA run ~8 pairs ahead.
            # Folds are deferred 2 pairs after their u-copy so PE never stalls.
            H = BATCH // 2
            blocks_lo = [(g, c0, min(c1, H)) for (g, c0, c1) in blocks if c0 < H]
            blocks_hi = [(g, max(c0, H), c1) for (g, c0, c1) in blocks if c1 > H]

            pending_fold = []
            for p in range(NPAIR):
                k0 = 2 * p
                ft = fpool.tile([128, 2, BATCH], DT, tag="flat")
                if p < NPAIR - 1:
                    nc.sync.dma_start(
                        out=ft, in_=flatt[k0:k0 + 2, :, :].rearrange("c p n -> p c n")
                    )
                else:
                    # last pair arrives in column halves so the tail copies can
                    # start on the first half while the second transfers
                    nc.sync.dma_start(
                        out=ft[:, :, 0:H],
                        in_=flatt[k0:k0 + 2, :, 0:H].rearrange("c p n -> p c n"),
                    )
                    nc.sync.dma_start(
                        out=ft[:, :, H:],
                        in_=flatt[k0:k0 + 2, :, H:].rearrange("c p n -> p c n"),
                    )
                if p == 0:
                    nc.sync.dma_start(out=wt[:, 0:16, :], in_=w[:, 0:16, :])
                    nc.sync.dma_start(out=rrt_sb[:, 0:4, :, :], in_=rrt[:, 0:4, :, :])
                elif p == 1:
                    nc.sync.dma_start(out=rrt_sb[:, 4:8, :, :], in_=rrt[:, 4:8, :, :])
                elif p == 2:
                    nc.sync.dma_start(out=wt[:, 16:32, :], in_=w[:, 16:32, :])
                    nc.sync.dma_start(out=ffd_sb, in_=ffd[:, :, :, :])
                elif p == 3:
                    nc.sync.dma_start(out=wt[:, 32:64, :], in_=w[:, 32:64, :])
                f1loc = k0 // 8              # this core's f1 index (0..7)
                blk = k0 % 8                 # rr block pair (blk, blk+1)
                pp = f1loc % 2               # u_ps ping-pong slot
                for half in range(2):
                    nc.tensor.matmul(
                        zcat_ps[:, half * H:(half + 1) * H],
                        wt[:, k0:k0 + 2, :],
                        ft[:, :, half * H:(half + 1) * H],
                        start=(p == 0),
                        stop=(p == NPAIR - 1),
                        perf_mode=DR,
                    )
                    # on the last pair, consume each column half right after
                    # its DMA half lands (enc then U per half)
                    half_blocks = (blocks_lo, blocks_hi)[half] \
                        if p == NPAIR - 1 else (blocks, ())[half]
                    for (g, c0, c1) in half_blocks:
                        nc.tensor.matmul(
                            u_ps[pp][:, c0:c1],
                            rrt_sb[:, blk:blk + 2, g, :],
                            ft[:, :, c0:c1],
                            start=(blk == 0),
                            stop=(blk == 6),
                            perf_mode=DR,
                        )
                if pending_fold and pending_fold[0][1] == p:
                    f1d, _, ppd = pending_fold.pop(0)
                    for (g, c0, c1) in blocks:
                        nc.tensor.matmul(
                            t_ps[:, c0:c1],
                            ffd_sb[:, f1d, g, :],
                            u_sb[ppd][:, c0:c1],
                            start=(f1d == 0),
                            stop=(f1d == F1_PER_CORE - 3),
                        )
                    if f1d == F1_PER_CORE - 3:
                        # t (f1 0..5) complete mid-stream: narrow it now, in
                        # halves on both engines. Its DMA is issued in the tail
                        # so the in-order sync queue never blocks later ft
                        # transfers.
                        t_sb = opool.tile([Z, BATCH], BF16, tag="t")
                        nc.vector.tensor_copy(t_sb[:, 0:H], t_ps[:, 0:H])
                        nc.scalar.copy(t_sb[:, H:], t_ps[:, H:])
                if blk == 6 and f1loc < F1_PER_CORE - 2:
                    # u for f1loc complete: narrow to bf16 (DVE/ACT alternate);
                    # the diag-Ff fold into t_ps is deferred 2 pairs.
                    if pp == 0:
                        nc.vector.tensor_copy(u_sb[pp], u_ps[pp])
                    else:
                        nc.scalar.copy(u_sb[pp], u_ps[pp])
                    pending_fold.append((f1loc, p + 2, pp))
                if blk == 6 and f1loc == F1_PER_CORE - 2:
                    # u6 ships raw (host folds Ff): narrow in halves now,
                    # DMA issued in the tail.
                    u6_sb = opool.tile([Z, BATCH], BF16, tag="u6")
                    nc.vector.tensor_copy(u6_sb[:, 0:H], u_ps[pp][:, 0:H])
                    nc.scalar.copy(u6_sb[:, H:], u_ps[pp][:, H:])

            # --- tail: u7 also ships raw. Copies are column-split, paired to
            # engines by readiness (enc halves gate zc, U halves gate u7).
            nc.sync.dma_start(out=t_out[:, :], in_=t_sb)
            nc.sync.dma_start(out=u6_out[:, :], in_=u6_sb)
            zc_sb = opool.tile([64, BATCH], BF16, tag="zc")
            u7_sb = opool.tile([Z, BATCH], BF16, tag="u7")
            nc.vector.tensor_copy(zc_sb[:, 0:H], zcat_ps[:, 0:H])
            nc.scalar.copy(u7_sb[:, 0:H], u_ps[1][:, 0:H])
            nc.vector.tensor_copy(zc_sb[:, H:], zcat_ps[:, H:])
            nc.scalar.copy(u7_sb[:, H:], u_ps[1][:, H:])
            nc.sync.dma_start(out=zcat_out[:, :], in_=zc_sb)
            nc.sync.dma_start(out=u7_out[:, :], in_=u7_sb)
    return nc


def _prepare(inputs):
    features = np.asarray(inputs["features"], dtype=np.float32)
    labels = np.asarray(inputs["labels"]).astype(np.int64)
    groups = np.asarray(inputs["groups"]).astype(np.int64)
    weights = np.asarray(inputs["weights"], dtype=np.float32)
    noise = np.asarray(inputs["noise"], dtype=np.float32)
    group_embed = np.asarray(inputs["group_embed"], dtype=np.float32)
    W1 = np.asarray(inputs["W1"], dtype=np.float32)
    b1 = np.asarray(inputs["b1"], dtype=np.float32)
    W2 = np.asarray(inputs["W2"], dtype=np.float32)
    b2 = np.asarray(inputs["b2"], dtype=np.float32)
    freq_factors = np.asarray(inputs["freq_factors"], dtype=np.float32)
    roi_1_factors = np.asarray(inputs["roi_1_factors"], dtype=np.float32)
    roi_2_factors = np.asarray(inputs["roi_2_factors"], dtype=np.float32)
    lin_W = np.asarray(inputs["lin_W"], dtype=np.float32)
    lin_b = np.asarray(inputs["lin_b"], dtype=np.float32)
    logit_bias = np.asarray(inputs["logit_bias"], dtype=np.float32)

    b = features.shape[0]
    flat = features.reshape(b, -1)

    perm = np.argsort(groups, kind="stable")
    groups_sorted = groups[perm]
    blocks = _group_blocks(groups_sorted)

    sq = np.einsum("bi,bi->b", flat, flat, optimize=True)

    flat_q = flat[perm].astype(NPDT)
    flatT = flat_q.view(np.uint8).T.copy().view(NPDT)       # [N_FEAT, BATCH]

    W = (np.concatenate([W1[:N_FEAT], W2[:N_FEAT]], axis=1) * W_SCALE).astype(NPDT)

    Ff = _softplus(freq_factors)             # [16, 32z, 64f1]
    R1 = _softplus(roi_1_factors)            # [16, 32z, 32r1]
    R2 = _softplus(roi_2_factors)            # [16, 32z, 32r2]

    # rrt[p=(dr1,r2), blk, g, z] = R1[g,z,4*blk+dr1] * R2[g,z,r2]
    A = R1.reshape(N_GROUPS, Z, 8, 4)                       # [g, z, blk, dr1]
    rr = A[:, :, :, :, None] * R2[:, :, None, None, :]      # [g, z, blk, dr1, r2]
    rrt = np.ascontiguousarray(
        rr.transpose(3, 4, 2, 0, 1).reshape(128, 8, N_GROUPS, Z)
    ).astype(NPDT)

    w_dev = W.view(np.uint8).reshape(N_CORES, KCHUNKS, 128, 64)
    w_dev = w_dev.transpose(0, 2, 1, 3).copy().view(NPDT)

    # ffd[c][z', f1loc, g, z] = (z'==z) * Ff[g, z, 8c + f1loc]  (diag fold)
    eye = np.eye(Z, dtype=np.float32)                       # [z', z]
    ffd_all = np.zeros((N_CORES, Z, F1_PER_CORE, N_GROUPS, Z),
                       dtype=ml_dtypes.bfloat16)
    for c in range(N_CORES):
        for f1loc in range(F1_PER_CORE):
            # [z', g, z] = eye[z', z] * Ff[g, z, 8c + f1loc]
            ffd_all[c, :, f1loc] = eye[:, None, :] * Ff[None, :, :, 8 * c + f1loc]

    in_maps = []
    for c in range(N_CORES):
        in_maps.append({
            "flatt": np.ascontiguousarray(
                flatT[c * FEAT_PER_CORE:(c + 1) * FEAT_PER_CORE].view(np.uint8)
            ).reshape(KCHUNKS, 128, BATCH).view(NPDT),
            "w": w_dev[c],
            "rrt": rrt,
            "ffd": ffd_all[c],
        })

    host = dict(
        labels=labels, groups=groups, weights=weights, noise=noise,
        group_embed=group_embed, W1=W1, b1=b1, W2=W2, b2=b2,
        lin_W=lin_W, lin_b=lin_b, logit_bias=logit_bias,
        Ff=Ff, R1=R1, R2=R2, sq=sq, perm=perm, b=b,
        groups_sorted=groups_sorted,
    )
    return in_maps, blocks, host


def _finish(zcatT, ttT, host):
    b = host["b"]
    perm = host["perm"]
    inv = np.empty_like(perm)
    inv[perm] = np.arange(b)

    zcat = (zcatT / W_SCALE).T[inv]                        # [b, 64]
    T = ttT.T[inv]                                         # [b, Z]

    groups = host["groups"]
    ge = host["group_embed"][groups]
    z_mu = zcat[:, :Z] + host["b1"] + ge @ host["W1"][N_FEAT:]
    z_log_std = zcat[:, Z:] + host["b2"] + ge @ host["W2"][N_FEAT:]
    sigma = EPSILON + np.exp(z_log_std)
    kld = np.sum(-np.log(sigma) + 0.5 * (sigma * sigma + z_mu * z_mu - 1.0), axis=1)
    zs = z_mu + sigma * host["noise"]
    zs = zs @ host["lin_W"] + host["lin_b"]
    a = _softplus(zs)

    Ff, R1, R2 = host["Ff"], host["R1"], host["R2"]
    M = (np.einsum("gzf,gyf->gzy", Ff, Ff)
         * np.einsum("gzr,gyr->gzy", R1, R1)
         * np.einsum("gzs,gys->gzy", R2, R2))
    vol2 = np.einsum("bz,bzy,by->b", a, M[groups], a)
    fdotv = np.sum(a * T, axis=1)
    rec = REG_STRENGTH * (host["sq"] - 2.0 * fdotv + vol2) / N_FEAT

    logits = np.concatenate([zs[:, :N_CLASSES - 1], np.ones((b, 1), np.float32)],
                            axis=1) + host["logit_bias"]
    m = logits.max(axis=1, keepdims=True)
    lse = m[:, 0] + np.log(np.exp(logits - m).sum(axis=1))
    log_probs = logits[np.arange(b), host["labels"]] - lse

    freq_loss = np.var(Ff, axis=0, ddof=1).mean(axis=1).sum()
    roi_loss = (np.var(R1, axis=0, ddof=1) + np.var(R2, axis=0, ddof=1)).mean(axis=1).sum()

    loss = np.mean(rec - host["weights"] * log_probs + KL_FACTOR * kld) \
        + freq_loss + roi_loss
    return np.float32(loss)


def kernel(**inputs) -> np.ndarray:
    _install_waitfix()
    in_maps, blocks, host = _prepare(inputs)
    nc = build_device_program(blocks)
    r = run_bass_kernel_spmd(nc, in_maps, core_ids=list(range(N_CORES)))
    zcatT = np.zeros((64, BATCH), np.float32)
    ttT = np.zeros((Z, BATCH), np.float32)
    # per-column Ff factors for each core's last two f1 (u6/u7 ship unfolded)
    f7 = host["Ff"][host["groups_sorted"]]            # [col, z, 64]
    for c in range(N_CORES):
        zcatT += np.asarray(r.results[c]["zcat"], dtype=np.float32)
        ttT += np.asarray(r.results[c]["t"], dtype=np.float32)
        ttT += f7[:, :, 8 * c + 6].T * np.asarray(r.results[c]["u6"],
                                                  dtype=np.float32)
        ttT += f7[:, :, 8 * c + 7].T * np.asarray(r.results[c]["u7"],
                                                  dtype=np.float32)
    return _finish(zcatT, ttT, host)
